# revision 44
# baseline (speedup 1.0000x reference)
"""Causal self-attention (B=4, T=2048, C=1024, H=16, D=64) on 8 TRN2 NeuronCores.

Sharding: core c handles batch b=c//2 and head-group g=c%2 (8 of 16 heads,
512 of 1024 feature columns). Each core projects q,k,v for its heads, runs
causal softmax(q k^T / sqrt(d)) v, and computes the partial o_proj
attn[:, cols] @ Wo[:, cols].T -> [T, C]. Host sums the two head-group
partials per batch and stacks batches.

v4 kernel design (vs the bf16 baseline: 244027ns -> 230818ns in the
TimelineSim cost model; rel err 1.24e-2 vs the 2e-2 gate):
  - q/k/v and o_proj matmuls run as fp8(e4m3) DoubleRow 3-term hi/lo
    products (hi*hi + hi*lo + lo*hi): K=256 per instruction at 0.5
    cycles/column gives 1.33x over bf16 at ~bf16 precision. Weights are
    pre-scaled x32 (and attn x8) on the host so both hi and lo splits sit
    in e4m3's normal range; the descales fold into the exp scale
    (2^-13), the softmax normalize (x0.25) and the output copy (x2^-8).
    Measured end-to-end rel err 3.8e-3, better than the all-bf16
    baseline's 5.3e-3 (fp8 hi+lo carries ~10 mantissa bits).
  - PV stays bf16 and scores stay bf16 for head-pairs j1-j3: single-fp8
    q or k on ALL heads measurably costs 2.4e-2 rel err on its own
    (softmax weights see the full e4m3 quantization noise), busting the
    2e-2 budget. Head-pair j0 alone spends the remaining budget: its
    scores run fp8 DoubleRow with k as hi/lo in the DR pair slots (full
    k precision) and q single-e4m3 duplicated across slots -- half-cost
    score matmuls for sqrt(1/4) of the full-fp8 noise (measured
    1.24e-2 end to end, matching the numpy error model to 4 digits).
  - scores run transposed per head pair as true K=64 matmuls from one
    combined kT tile (rows 0:64 even-head d's, 64:128 odd): cost is
    moving columns only. The odd heads run at tile_position (64,0); e/o
    outputs land in different PSUM banks so positions never share one.
  - exp batches TWO kt-blocks x both parities per activation (1024
    elems), halving Act instruction overhead (the Act engine is the
    secondary bottleneck: ~151us busy vs PE ~176us).
  - PSUM (8 banks): sp 2 bufs x 2 banks, pv 2 banks (16 tq-half x
    head accumulators of 64 cols), proj/o_proj/transpose scratch 2
    one-bank buffers. start_tensor_calc pending-zero is BANK-wide, so
    the scratch ping-pong must span two banks or every group serializes
    against the previous group's drain copy (the Tile scheduler models
    the bank-wide write). pv banks are opened once per chunk with
    1-column start=True matmuls; everything then accumulates
    start=False (first write to a pending byte replaces).
  - softmax denominators: [128,1] ones-matmuls reusing the PV lhsT,
    batched at the chunk epilogue into a just-freed score bank (the sp
    ring slot the next chunk's second pair will overwrite); pt tiles
    are retained through the chunk (pool of 36) to feed them.
  - per-chunk pipeline: score-pair -> exp -> (lag 4) PV, with
    next-chunk projections (q-projections FIRST: the next chunk's first
    score pairs need only qT; new k/v columns feed only the late
    diagonal pair and final pend-drain) and backlogged o_proj groups
    interleaved uniformly between score pairs; transposes + fp8 hi/lo
    splits for chunk c-1 run inside chunk c's stream.
"""

import numpy as np

B, T, C, H, D = 4, 2048, 1024, 16, 64
NCORES = 8
FH = 512          # features per core = 8 heads
NCP = 4           # contraction-tile pairs (K=1024 = 4 x 256)
JP = 4            # head-pair tiles (8 heads / 2)
CH = 256          # tq chunk width
NCH = T // CH     # 8 chunks

_CACHE = {}


def _build():
    import concourse.bass as bass  # noqa: F401
    import concourse.mybir as mybir
    from concourse import bacc
    from concourse.tile import TileContext

    F32 = mybir.dt.float32
    F32R = mybir.dt.float32r
    BF16 = mybir.dt.bfloat16
    F8 = mybir.dt.float8e4
    EXP = mybir.ActivationFunctionType.Exp
    DR = mybir.MatmulPerfMode.DoubleRow
    MUL = mybir.AluOpType.mult
    SUB = mybir.AluOpType.subtract

    nc = bacc.Bacc("TRN2", target_bir_lowering=False, debug=False, num_devices=NCORES)
    xh_h = nc.dram_tensor("xh", [128, NCP, 2, T], F8, kind="ExternalInput")
    xl_h = nc.dram_tensor("xl", [128, NCP, 2, T], F8, kind="ExternalInput")
    wqh_h = nc.dram_tensor("wqh", [128, NCP, 2, FH], F8, kind="ExternalInput")
    wql_h = nc.dram_tensor("wql", [128, NCP, 2, FH], F8, kind="ExternalInput")
    wkh_h = nc.dram_tensor("wkh", [128, NCP, 2, FH], F8, kind="ExternalInput")
    wkl_h = nc.dram_tensor("wkl", [128, NCP, 2, FH], F8, kind="ExternalInput")
    wvh_h = nc.dram_tensor("wvh", [128, NCP, 2, FH], F8, kind="ExternalInput")
    wvl_h = nc.dram_tensor("wvl", [128, NCP, 2, FH], F8, kind="ExternalInput")
    woh_h = nc.dram_tensor("woh", [128, 2, 2, C], F8, kind="ExternalInput")
    wol_h = nc.dram_tensor("wol", [128, 2, 2, C], F8, kind="ExternalInput")
    out_h = nc.dram_tensor("out", [T, C], F32, kind="ExternalOutput")
    out_ap = out_h.ap()

    with TileContext(nc) as tc:
        with (
            tc.tile_pool(name="persist", bufs=1) as persist,
            tc.tile_pool(name="qp", bufs=2) as qp,
            tc.tile_pool(name="qp8", bufs=2) as qp8,
            tc.tile_pool(name="ptp", bufs=36) as ptp,
            tc.tile_pool(name="asbp", bufs=2) as asbp,
            tc.tile_pool(name="athp", bufs=2) as athp,
            tc.tile_pool(name="atlp", bufs=2) as atlp,
            tc.tile_pool(name="opl", bufs=3) as opool,
            tc.tile_pool(name="rp", bufs=2) as rp,
            tc.tile_pool(name="spp", bufs=2, space="PSUM") as spp,
            tc.tile_pool(name="pvp", bufs=1, space="PSUM") as pvp,
            tc.tile_pool(name="ppp", bufs=2, space="PSUM") as ppp,
        ):
            xh = persist.tile([128, NCP, 2, T], F8, tag="xh")
            xl = persist.tile([128, NCP, 2, T], F8, tag="xl")
            wqh = persist.tile([128, NCP, 2, FH], F8, tag="wqh")
            wql = persist.tile([128, NCP, 2, FH], F8, tag="wql")
            wkh = persist.tile([128, NCP, 2, FH], F8, tag="wkh")
            wkl = persist.tile([128, NCP, 2, FH], F8, tag="wkl")
            wvh = persist.tile([128, NCP, 2, FH], F8, tag="wvh")
            wvl = persist.tile([128, NCP, 2, FH], F8, tag="wvl")
            woh = persist.tile([128, 2, 2, C], F8, tag="woh")
            wol = persist.tile([128, 2, 2, C], F8, tag="wol")
            # k^T for both head parities in one tile: rows 0:64 even-head
            # d's, rows 64:128 odd-head d's. Scores run as true K=64
            # matmuls (cost is moving columns, K doesn't matter) with the
            # odd heads at tile_position (64, 0); the e/o score outputs go
            # to different PSUM banks so the tile positions never share one.
            kT2 = persist.tile([128, JP, T], BF16, tag="kT2")
            # head-pair j0 runs its scores in fp8 DoubleRow (k as hi/lo in
            # the DR slots -> only q carries e4m3 quantization noise; that
            # spends ~1.2e-2 of the 2e-2 budget for half-cost score matmuls)
            kT8 = persist.tile([128, 2, T], F8, tag="kT8")
            v_s = persist.tile([128, 2 * NCH, 8, D], BF16, tag="vs")

            # startup DMAs in critical-path order: wk + x chunk0 gate the
            # k-projection, then wq, wv, remaining x chunks, wo.
            def load_x(c):
                sl = (slice(None), slice(None), slice(None),
                      slice(c * CH, (c + 1) * CH))
                nc.sync.dma_start(out=xh[sl], in_=xh_h.ap()[sl])
                nc.sync.dma_start(out=xl[sl], in_=xl_h.ap()[sl])

            nc.sync.dma_start(out=wkh, in_=wkh_h.ap())
            nc.sync.dma_start(out=wkl, in_=wkl_h.ap())
            load_x(0)
            nc.sync.dma_start(out=wqh, in_=wqh_h.ap())
            nc.sync.dma_start(out=wql, in_=wql_h.ap())
            load_x(1)
            nc.sync.dma_start(out=wvh, in_=wvh_h.ap())
            nc.sync.dma_start(out=wvl, in_=wvl_h.ap())
            load_x(2)
            nc.sync.dma_start(out=woh, in_=woh_h.ap())
            nc.sync.dma_start(out=wol, in_=wol_h.ap())
            for cc in range(3, NCH):
                load_x(cc)

            # proj/o_proj/transpose scratch: two one-bank buffers.
            # start_tensor_calc pending-zero is BANK-wide, so the two
            # ping-pong slots must sit in different banks or every group
            # serializes against the previous group's drain copy.
            _ppn = [0]
            _pp_extra = []  # extra tail slots (freed score banks)

            def pp_tile():
                _ppn[0] += 1
                if _pp_extra and _ppn[0] % 2 == 0:
                    _pp_extra.append(_pp_extra.pop(0))
                    return _pp_extra[-1]
                return ppp.tile([128, 256], F32, tag="pp", name=f"pp{_ppn[0]}")

            onesb = persist.tile([128, 1], BF16, tag="ones")
            nc.gpsimd.memset(onesb, 1.0)
            z128 = persist.tile([128, 128], BF16, tag="z128")
            nc.gpsimd.memset(z128, 0.0)
            # identity for PE transposes
            idn = persist.tile([128, 128], F32, tag="idn")
            nc.gpsimd.memset(idn, 1.0)
            nc.gpsimd.affine_select(
                out=idn, in_=idn, compare_op=mybir.AluOpType.is_ge, fill=0.0,
                base=0, pattern=[[1, 128]], channel_multiplier=-1,
            )
            nc.gpsimd.affine_select(
                out=idn, in_=idn, compare_op=mybir.AluOpType.is_ge, fill=0.0,
                base=0, pattern=[[-1, 128]], channel_multiplier=1,
            )

            def proj3(ps, wh, wl, ncols, wcol0, xcol0, first_extra=None):
                """12 DR matmuls: (x_hi+x_lo)@(W_hi+W_lo) minus lo*lo.
                ps: [128, ncols] psum; w slices [128, NCP, 2, .]; x cols."""
                n = 0
                for cp in range(NCP):
                    for wt, xt in ((wh, xh), (wh, xl), (wl, xh)):
                        nc.tensor.matmul(
                            ps,
                            wt[:, cp, :, wcol0:wcol0 + 128],
                            xt[:, cp, :, xcol0:xcol0 + ncols],
                            start=(n == 0), stop=(n == 3 * NCP - 1),
                            skip_group_check=True, perf_mode=DR,
                        )
                        n += 1
                        yield 54

            def vproj3(ps, tt0, fcol0):
                n = 0
                for cp in range(NCP):
                    for at, bt in ((xh, wvh), (xh, wvl), (xl, wvh)):
                        nc.tensor.matmul(
                            ps,
                            at[:, cp, :, tt0:tt0 + 128],
                            bt[:, cp, :, fcol0:fcol0 + 256],
                            start=(n == 0), stop=(n == 3 * NCP - 1),
                            skip_group_check=True, perf_mode=DR,
                        )
                        n += 1
                        yield 54

            def k_steps(c, j, wsrc=None):
                ps = pp_tile()
                wh, wl = wsrc or (wkh, wkl)
                yield from proj3(ps, wh, wl, CH, j * 128, c * CH)
                cols = slice(c * CH, (c + 1) * CH)
                if j == 0:
                    nc.vector.tensor_copy(out=kT8[:, 0, cols], in_=ps)
                    nc.vector.scalar_tensor_tensor(
                        out=kT8[:, 1, cols], in0=ps, scalar=1.0,
                        in1=kT8[:, 0, cols], op0=MUL, op1=SUB,
                    )
                else:
                    nc.vector.tensor_copy(out=kT2[0:64, j, cols],
                                          in_=ps[0:64, :])
                    nc.vector.tensor_copy(out=kT2[64:128, j, cols],
                                          in_=ps[64:128, :])
                yield 0

            def q_steps(c, j, qT_t, qT8_t, wsrc=None):
                ps = pp_tile()
                wh, wl = wsrc or (wqh, wql)
                yield from proj3(ps, wh, wl, CH, j * 128, c * CH)
                if j == 0:
                    nc.vector.tensor_copy(out=qT8_t[:, 0, :], in_=ps)
                    nc.vector.tensor_copy(out=qT8_t[:, 1, :], in_=ps)
                else:
                    nc.vector.tensor_copy(out=qT_t[:, j, :], in_=ps)
                yield 0

            def v_steps(c, tt, fh):
                ps = pp_tile()
                yield from vproj3(ps, c * CH + tt * 128, fh * 256)
                nc.vector.tensor_copy(
                    out=v_s[:, 2 * c + tt, 4 * fh:4 * fh + 4, :],
                    in_=ps.rearrange("p (h d) -> p h d", h=4),
                )
                yield 0

            def kqv_steps(c, qT_t, qT8_t, q_first=True):
                # q first: the next chunk's first score pairs only need qT
                # (k's new columns feed only the late diagonal pair, v's new
                # rows only the final pend-drain), so the boundary dependency
                # chain collapses to the first q group.
                if q_first:
                    for j in range(JP):
                        yield from q_steps(c, j, qT_t, qT8_t)
                    for j in range(JP):
                        yield from k_steps(c, j)
                else:
                    for j in range(JP):
                        yield from k_steps(c, j)
                        yield from q_steps(c, j, qT_t, qT8_t)
                for fh in range(2):
                    for tt in range(2):
                        yield from v_steps(c, tt, fh)

            def o_group(c, ath, atl, mt, n, act_copy=False):
                po = pp_tile()
                i = 0
                for a_t, w_t in ((ath, woh), (ath, wol), (atl, woh)):
                    for fp in range(2):
                        nc.tensor.matmul(
                            po,
                            a_t[:, fp, :, mt * 128:(mt + 1) * 128],
                            w_t[:, fp, :, n * 256:(n + 1) * 256],
                            start=(i == 0), stop=(i == 5),
                            skip_group_check=True, perf_mode=DR,
                        )
                        i += 1
                        yield 54
                ot = opool.tile([128, 256], F32, tag="ot")
                if act_copy:
                    # tail-only: Act is idle there, DVE is the tail floor
                    nc.scalar.mul(ot, po, 2.0 ** -8)
                else:
                    nc.vector.tensor_scalar_mul(ot, po, 2.0 ** -8)
                nc.sync.dma_start(
                    out=out_ap[c * CH + mt * 128: c * CH + (mt + 1) * 128,
                               n * 256:(n + 1) * 256],
                    in_=ot,
                )
                yield 0

            def tr_steps(asb2, ath, atl):
                # transpose attn [tq, f] -> attnT [f, tq] (PE, identity),
                # then split to fp8 hi/lo for the DR o_proj
                for fb in range(4):
                    tp = pp_tile()
                    for a in range(2):
                        nc.tensor.transpose(
                            tp[:, a * 128:(a + 1) * 128],
                            asb2[:, a, fb * 128:(fb + 1) * 128], idn,
                        )
                        yield 107
                    dh = ath[:, fb // 2, fb % 2, :]
                    nc.vector.tensor_copy(out=dh, in_=tp)
                    nc.vector.scalar_tensor_tensor(
                        out=atl[:, fb // 2, fb % 2, :], in0=tp, scalar=1.0,
                        in1=dh, op0=MUL, op1=SUB,
                    )
                    yield 0

            def chain(*gens):
                for g in gens:
                    yield from g

            SENT = object()

            # ---- prologue: k, q, v projections for chunk 0 ----
            qT_cur = qp.tile([128, JP, CH], BF16, tag="qT")
            qT8_cur = qp8.tile([128, 2, CH], F8, tag="qT8")
            for j in range(JP):
                for _ in k_steps(0, j):
                    pass
                for _ in q_steps(0, j, qT_cur, qT8_cur):
                    pass
            for fh in range(2):
                for tt in range(2):
                    for _ in v_steps(0, tt, fh):
                        pass

            pending_tr = None
            backlog = []  # deferred o_proj groups: (birth_chunk, gen)

            for c in range(NCH):
                # open pv banks as single accumulation groups
                pv0 = pvp.tile([128, 2, 4, D], F32, tag="pv0", name=f"pv0_{c}")
                pv1 = pvp.tile([128, 2, 4, D], F32, tag="pv1", name=f"pv1_{c}")
                pv = (pv0, pv1)
                nc.tensor.matmul(pv0[:, 0, 0, 0:1], z128, onesb,
                                 start=True, stop=False, skip_group_check=True)
                nc.tensor.matmul(pv1[:, 0, 0, 0:1], z128, onesb,
                                 start=True, stop=False, skip_group_check=True)
                pts = {}

                def emit_pv(j, m, pt):
                    # pv + den matmuls for score pair m of head-pair j
                    # pt dims: [128 tk, kt-slot, parity, tq]
                    for hh in range(2):
                        h = 2 * j + hh
                        x, hx = (0, h) if h < 4 else (1, h - 4)
                        for s in range(2):
                            kt = 2 * m + s
                            for a in range(2):
                                ig = 2 * c + a
                                if kt > ig:
                                    continue
                                lhs = pt[:, hh, s, a * 128:(a + 1) * 128]
                                nc.tensor.matmul(
                                    pv[a][:, x, hx, :], lhs, v_s[:, kt, h, :],
                                    start=False, stop=(kt == ig),
                                    skip_group_check=True,
                                )

                n_pairs = JP * (c + 1)
                gens = []
                est_ns = 0.0
                # force-drain backlog older than one chunk (attnT ring depth)
                while backlog and backlog[0][0] <= c - 2:
                    gens.append(backlog.pop(0)[1])
                    est_ns += 380.0
                if c + 1 < NCH:
                    qT_next = qp.tile([128, JP, CH], BF16, tag="qT")
                    qT8_next = qp8.tile([128, 2, CH], F8, tag="qT8")
                    gens.append(kqv_steps(c + 1, qT_next, qT8_next))
                    est_ns += 7800.0
                if pending_tr is not None:
                    # transposes + hi/lo splits for chunk c-1 run mid-chunk,
                    # away from the epilogue's DVE burst
                    gens.append(tr_steps(*pending_tr))
                    est_ns += 900.0
                    prev_ath, prev_atl = pending_tr[1], pending_tr[2]
                    for mt in range(2):
                        for n in range(4):
                            backlog.append((c - 1, o_group(c - 1, prev_ath,
                                                          prev_atl, mt, n)))
                # drain backlog into chunks with enough score pairs to hide it
                quota = max(0, n_pairs - 14)
                while backlog and quota > 0:
                    gens.append(backlog.pop(0)[1])
                    est_ns += 380.0
                    quota -= 1
                stream = chain(*gens)
                # spread deferred PE work uniformly across the score pairs
                per_pair = est_ns / n_pairs
                emitted = 0.0
                want = 0.0

                pend = []
                for j in range(JP):
                    for m in range(c + 1):
                        sp = spp.tile([128, 2, 2, CH], F32, tag="s")
                        diag = (m == c)
                        c0 = 128 if diag else 0
                        # sp dims: [128, parity(bank), kt-slot, tq]; the
                        # diagonal pair's second kt computes only cols 128:
                        for s in range(2):
                            s0 = c0 if s == 1 else 0
                            kts = slice((2 * m + s) * 128,
                                        (2 * m + s + 1) * 128)
                            if j == 0:
                                # fp8 DR: k hi/lo in the pair slots, q8 dup
                                nc.tensor.matmul(
                                    sp[:, 0, s, s0:CH], kT8[0:64, :, kts],
                                    qT8_cur[0:64, :, s0:CH],
                                    start=True, stop=True, perf_mode=DR,
                                )
                                nc.tensor.matmul(
                                    sp[:, 1, s, s0:CH], kT8[64:128, :, kts],
                                    qT8_cur[64:128, :, s0:CH],
                                    start=True, stop=True, perf_mode=DR,
                                )
                            else:
                                nc.tensor.matmul(
                                    sp[:, 0, s, s0:CH], kT2[0:64, j, kts],
                                    qT_cur[0:64, j, s0:CH],
                                    start=True, stop=True,
                                )
                                nc.tensor.matmul(
                                    sp[:, 1, s, s0:CH], kT2[64:128, j, kts],
                                    qT_cur[64:128, j, s0:CH],
                                    start=True, stop=True,
                                )
                        pt = ptp.tile([128, 2, 2, CH], BF16, tag="pt")
                        # exp over both kt-blocks and parities at once; the
                        # diag pair's [.,1,.,0:128] quarter exps stale psum
                        # (finite, never read downstream). pt dims match sp:
                        # [128, kt-slot, parity, tq]
                        nc.scalar.activation(
                            out=pt, in_=sp, func=EXP, scale=2.0 ** -13,
                        )
                        if diag:
                            # zero upper triangle of the two diagonal blocks
                            nc.gpsimd.affine_select(
                                out=pt[:, :, 0, 0:128], in_=pt[:, :, 0, 0:128],
                                compare_op=mybir.AluOpType.is_ge, fill=0.0,
                                base=0, pattern=[[0, 2], [1, 128]],
                                channel_multiplier=-1,
                            )
                            nc.gpsimd.affine_select(
                                out=pt[:, :, 1, 128:CH], in_=pt[:, :, 1, 128:CH],
                                compare_op=mybir.AluOpType.is_ge, fill=0.0,
                                base=0, pattern=[[0, 2], [1, 128]],
                                channel_multiplier=-1,
                            )
                        pts[(j, m)] = pt
                        pend.append((j, m, pt))
                        if len(pend) > 4:
                            emit_pv(*pend.pop(0))
                        want += per_pair
                        while emitted < want:
                            r = next(stream, SENT)
                            if r is SENT:
                                emitted = float("inf")
                                break
                            emitted += r
                for _ in stream:
                    pass
                for e in pend:
                    emit_pv(*e)

                # epilogue: denominators batched into a just-freed score
                # bank (the sp ring tile the NEXT chunk's second pair will
                # reuse), then normalize on DVE
                den_sp = spp.tile([128, 2, 2, CH], F32, tag="s")
                den = den_sp[:, 0, 0, 0:16].rearrange(
                    "p (a x h) -> p a x h", a=2, x=2)
                first = True
                for j in range(JP):
                    for m in range(c + 1):
                        ptt = pts[(j, m)]
                        for hh in range(2):
                            h = 2 * j + hh
                            x, hx = (0, h) if h < 4 else (1, h - 4)
                            for s in range(2):
                                kt = 2 * m + s
                                for a in range(2):
                                    ig = 2 * c + a
                                    if kt > ig:
                                        continue
                                    nc.tensor.matmul(
                                        den[:, a, x, hx:hx + 1],
                                        ptt[:, hh, s, a * 128:(a + 1) * 128],
                                        onesb, start=first,
                                        stop=(kt == ig),
                                        skip_group_check=True,
                                    )
                                    first = False
                rec = rp.tile([128, 2, 2, 4], F32, tag="rec")
                nc.vector.reciprocal(out=rec, in_=den)
                asb = asbp.tile([128, 2, 8, D], F32, tag="asb")
                for a in range(2):
                    for x in range(2):
                        nc.vector.scalar_tensor_tensor(
                            out=asb[:, a, 4 * x:4 * x + 4], in0=pv[a][:, x],
                            scalar=0.25,
                            in1=rec[:, a, x, :, None].broadcast_to([128, 4, D]),
                            op0=MUL, op1=MUL,
                        )
                asb2 = asb.rearrange("p a h d -> p a (h d)")
                ath = athp.tile([128, 2, 2, CH], F8, tag="ath")
                atl = atlp.tile([128, 2, 2, CH], F8, tag="atl")
                pending_tr = (asb2, ath, atl)
                if c + 1 < NCH:
                    qT_cur = qT_next
                    qT8_cur = qT8_next

            # tail: leftover backlog, then transposes + o_proj for the
            # last chunk. The score banks are free now — widen the scratch
            # rotation to 6 banks and put the output copies on the (idle)
            # Act engine so the tail chain pipelines.
            for _, g in backlog:
                for _ in g:
                    pass
            for _ in tr_steps(*pending_tr):
                pass
            for mt in range(2):
                for n in range(4):
                    for _ in o_group(NCH - 1, pending_tr[1], pending_tr[2],
                                     mt, n):
                        pass

    nc.compile()
    return nc


def _get_nc():
    if "nc" not in _CACHE:
        _CACHE["nc"] = _build()
    return _CACHE["nc"]


def _split8(a):
    import ml_dtypes

    f8 = ml_dtypes.float8_e4m3
    hi = np.ascontiguousarray(a).astype(f8)
    lo = (a - hi.astype(np.float32)).astype(f8)
    return hi, lo


def _pack_w(w):
    # [C, F] -> [128, NCP, 2, F] with c = cp*256 + t*128 + p
    Cdim, F = w.shape
    return np.ascontiguousarray(
        w.reshape(Cdim // 256, 2, 128, F).transpose(2, 0, 1, 3)
    )


def make_in_maps(x, Wq, Wk, Wv, Wo):
    x = np.asarray(x, dtype=np.float32)
    Wq = np.asarray(Wq, dtype=np.float32)
    Wk = np.asarray(Wk, dtype=np.float32)
    Wv = np.asarray(Wv, dtype=np.float32)
    Wo = np.asarray(Wo, dtype=np.float32)
    in_maps = []
    for core in range(NCORES):
        b, g = core // 2, core % 2
        cols = slice(FH * g, FH * (g + 1))
        # x^T [C, T] split to fp8 hi/lo, packed [128, NCP, 2, T]
        xT = np.ascontiguousarray(x[b].T)
        xhi, xlo = _split8(xT)

        def xpack(a):
            return np.ascontiguousarray(
                a.reshape(NCP, 2, 128, T).transpose(2, 0, 1, 3)
            )

        m = {"xh": xpack(xhi), "xl": xpack(xlo)}
        for name, W in (("wq", Wq), ("wk", Wk), ("wv", Wv)):
            Wt = np.ascontiguousarray(W.T[:, cols]) * 32.0
            hi, lo = _split8(Wt)
            m[name + "h"] = _pack_w(hi)
            m[name + "l"] = _pack_w(lo)
        # Wo^T [FH, C] x32, f = fp*256 + t*128 + p -> [128, 2, 2, C]
        WoT = np.ascontiguousarray(Wo.T[cols, :]) * 32.0
        whi, wlo = _split8(WoT)

        def wopack(a):
            return np.ascontiguousarray(
                a.reshape(2, 2, 128, C).transpose(2, 0, 1, 3)
            )

        m["woh"] = wopack(whi)
        m["wol"] = wopack(wlo)
        in_maps.append(m)
    return in_maps


def gather_out(parts):
    return np.stack([parts[2 * b] + parts[2 * b + 1] for b in range(B)])


def kernel(x, Wq, Wk, Wv, Wo):
    from concourse.bass_utils import run_bass_kernel_spmd

    nc = _get_nc()
    in_maps = make_in_maps(x, Wq, Wk, Wv, Wo)
    try:
        res = run_bass_kernel_spmd(nc, in_maps, core_ids=list(range(NCORES)))
    except Exception:
        # transient NRT device errors have been observed on this fabric;
        # one retry costs nothing when healthy
        res = run_bass_kernel_spmd(nc, in_maps, core_ids=list(range(NCORES)))
    return gather_out([res.results[c]["out"] for c in range(NCORES)])
